# revision 77
# baseline (speedup 1.0000x reference)
# Trainium2 Bass kernel for nn_CvtLstm: ConvLSTM cell with 4-branch,
# 4-head spatial attention. Data-parallel over batch N=32 across 8
# NeuronCores (4 samples per core); weights replicated to every core.
#
# Per-core layout: channels on partitions, flattened 16x16 spatial (256)
# on the free dim. conv3x3 = 9 shifted matmuls reading a zero-padded
# [128, 2, 18, 18] tile. Attention scores are computed directly in the
# transposed [d, q] layout (lhsT = per-head k rows, K=32 row-partial
# matmuls); exp on the ACT engine with no max subtraction (scores lie in
# [-9, 8]), written as bf16; the PV product and the softmax denominator Z
# come from one M=64 matmul per (head, d-chunk) whose bf16 weight columns
# are [vT_g | ones]. Matmul outputs must start at partition 0, so heads
# are separated via a DMA restack batched over norm groups (4 iterations
# early, 2 at the end: 8 DMAs + one batched reciprocal + one multiply per
# group). Each AZ PSUM->SBUF drain is split across DVE and ACT.
#
# The Tile framework list-schedules instructions by readiness with
# emission order as priority, so the emission schedule is shaped to give
# it slack: pvz matmuls for iteration i are interleaved into the score
# row-group boundaries of iteration i+2 (full-row separators for the
# row-partial score matmuls, and a full iteration window for the AZ
# drain); prep work (convs / qk / vt projections) is queued as
# self-contained matmul+copy units drained through those boundaries; the
# last two iterations' PV outputs borrow the then-dead ST buffers; gate
# matmul operand order makes each gate accumulate as its norm group
# lands, with four concurrent PSUM accumulators for the final gates; a
# few dependency-pinned warmup matmuls hold the PE p-state through the
# final norm chain.
#
# Gate sigmoids are computed as sigmoid(x) = 0.5*(1 + tanh(x/2)) so every
# activation (tanh/exp) lives in one ACT function table -- no table swaps.
# All weights arrive in one packed DRAM blob via 4 chunked DMAs (a tiny
# first chunk unblocks the input projection immediately) and every input
# DMA is issued before any compute-dependent DMA so the in-order SP queue
# never head-of-line blocks. Zero padding and the vt ones-columns are
# written by Pool-engine memsets. Elementwise work is spread over
# Pool/DVE/ACT to keep each engine under the PE roofline.
# TimelineSim: 103.7 us/core (baseline of this kernel's ancestor: 141.9).

import numpy as np

N, I, H, W = 32, 64, 16, 16
R, CM, A, HEADS, HC = 128, 128, 128, 4, 32
HW = H * W           # 256
S = 4                # samples per core
NCORES = 8

# weight blob column offsets
OFF_WIN = 0          # [64p, 128] (rows 64:128 zero)
OFF_BIN = 128        # [128, 1]
OFF_CONVH = 129      # [128, 9*128] W_ch taps
OFF_CONVX = OFF_CONVH + 1152   # [128, 9*128] W_cx taps
OFF_QK = OFF_CONVX + 1152      # [128, 2*4*128] (q|k, branch)
OFF_WV = OFF_QK + 1024         # [128, 2*256] (src, branch-pair)
OFF_ONES = OFF_WV + 512        # [128, 32]
OFF_TOK = OFF_ONES + 32        # [128, 4*4*128] (gate, branch)
OFF_SKIP = OFF_TOK + 2048      # [128, 4*2*128] (gate, src)
OFF_WOUT = OFF_SKIP + 1024     # [128, 128]
OFF_BTOK = OFF_WOUT + 128      # [128, 4] (pre-scaled 0.5 except gate 2)
OFF_BOUT = OFF_BTOK + 4        # [128, 1]
NWCOL = OFF_BOUT + 1

# chunk boundaries for the 4 weight DMAs (ordered by first use)
WCH0 = OFF_CONVH            # winT + b_in (tiny -- unblocks XT fast)
WCH1 = OFF_CONVX            # conv_h
WCH2 = OFF_TOK              # conv_x + qk + wv + ones
WCH3 = NWCOL                # tok + skip + wout + btok + bout

_CACHE = {}


def _build_program():
    import contextlib
    import concourse.bacc as bacc
    import concourse.mybir as mybir
    import concourse.tile as tile
    import concourse.bass as bass

    F32 = mybir.dt.float32
    F32R = mybir.dt.float32r
    BF16 = mybir.dt.bfloat16
    AF = mybir.ActivationFunctionType
    ALU = mybir.AluOpType

    nc = bacc.Bacc("TRN2", target_bir_lowering=False, debug=False)

    def dram(name, shape, kind="ExternalInput"):
        return nc.dram_tensor(name, list(shape), F32, kind=kind).ap()

    xin = dram("xin", [S, I, HW])
    hin = dram("hin", [S, R, HW])
    cin = dram("cin", [S, R, HW])
    wblob = dram("wblob", [128, NWCOL])
    yout = dram("yout", [S, R, HW], kind="ExternalOutput")

    QSRC = [0, 0, 1, 1]   # q source per branch: 0=xc, 1=hc
    KSRC = [0, 1, 0, 1]   # k/v source per branch
    BORDER = [3, 1, 2, 0]  # per-pass branch order (b3 = pure hc, earliest)

    with tile.TileContext(nc) as tc:
        with contextlib.ExitStack() as ctx:
            wpool = ctx.enter_context(tc.tile_pool(name="wts", bufs=1))
            sbA = ctx.enter_context(tc.tile_pool(name="sbA", bufs=2))
            sbB = ctx.enter_context(tc.tile_pool(name="sbB", bufs=2))
            sbN = ctx.enter_context(tc.tile_pool(name="sbN", bufs=1))
            ptp = ctx.enter_context(tc.tile_pool(name="ptp", bufs=3))
            afp = ctx.enter_context(tc.tile_pool(name="afp", bufs=2))
            stp = ctx.enter_context(tc.tile_pool(name="st", bufs=2, space="PSUM"))
            azp = ctx.enter_context(tc.tile_pool(name="az", bufs=1, space="PSUM"))
            pwp = ctx.enter_context(tc.tile_pool(name="pw", bufs=2, space="PSUM"))

            # ---------------- weights to SBUF (one tile, 4 DMAs) ----------
            # tiny chunk 0 (win+b_in) first so XT can start immediately;
            # remaining chunks interleaved with the input DMAs below.
            wb = wpool.tile([128, NWCOL], F32R, tag="wb", name="wb")
            nc.sync.dma_start(out=wb[:, 0:WCH0],
                              in_=wblob[:, 0:WCH0].bitcast(F32R))

            def wcols(off, n):
                return wb[:, off:off + n]

            ones32 = wcols(OFF_ONES, 32)
            b_in_ap = wcols(OFF_BIN, 1).bitcast(F32)
            btok_ap = wcols(OFF_BTOK, 4).bitcast(F32)
            bout_ap = wcols(OFF_BOUT, 1).bitcast(F32)

            # ---------------- per-pass state ----------------
            pad_sb = [None, None]      # [128, 2, 648]: 0=xt pad, 1=h pad
            xh_sb = [None, None]       # [128, 1024]: xc | hc
            q_sb = [[None] * 4, [None] * 4]
            k_sb = [[None] * 4, [None] * 4]
            vt_sb = [[None] * 4, [None] * 4]   # per branch: [128, 4, 4, 64]
            a_all = [None, None]
            cprev_sb = [None, None]
            gate_sb = [[None] * 4, [None] * 4]

            x2_sb = [None, None]

            def emit_input_dmas(p):
                """zero-padded xt/h pads (Pool memset) + h/x input DMAs."""
                pad = sbA.tile([128, 2, 648], F32R, tag="padc", name="pad")
                pad_sb[p] = pad
                nc.gpsimd.memset(pad.bitcast(F32), 0.0)
                x2_sb[p] = sbA.tile([64, 2, 256], F32R, tag="x2", name="x2")
                hv = pad[:, 1, :].rearrange("p (s y x) -> p s y x", s=2, y=18, x=18)
                for s in range(2):
                    nc.sync.dma_start(
                        out=hv[:, s, 1:17, 1:17],
                        in_=hin[2 * p + s].rearrange(
                            "c (h w) -> c h w", h=16).bitcast(F32R))
                nc.sync.dma_start(
                    out=x2_sb[p],
                    in_=xin[2 * p:2 * p + 2].rearrange("s c q -> c s q").bitcast(F32R))

            def emit_xt(p):
                """XT matmul + tanh into the xt pad."""
                pad = pad_sb[p]
                XT = pwp.tile([128, 512], F32, tag="pw", name="XT")
                nc.tensor.matmul(out=XT, lhsT=wb[0:64, OFF_WIN:OFF_WIN + 128],
                                 rhs=x2_sb[p].rearrange("p s q -> p (s q)"),
                                 start=True, stop=True)
                xv = pad[:, 0, :].rearrange("p (s y x) -> p s y x", s=2, y=18, x=18)
                nc.scalar.activation(
                    out=xv[:, :, 1:17, 1:17],
                    in_=XT.rearrange("p (s h w) -> p s h w", s=2, h=16, w=16),
                    func=AF.Tanh, bias=b_in_ap)

            def emit_conv(p, src):
                """3x3 SAME conv via 9 shifted matmuls; src 0=xc, 1=hc."""
                CP = pwp.tile([128, 512], F32, tag="pw", name="CP")
                pv = pad_sb[p][:, src, :].rearrange(
                    "p (s y x) -> p s y x", s=2, y=18, x=18)
                woff = OFF_CONVX if src == 0 else OFF_CONVH
                for t in range(9):
                    ky, kx = divmod(t, 3)
                    nc.tensor.matmul(out=CP, lhsT=wcols(woff + t * 128, 128),
                                     rhs=pv[:, :, ky:ky + 16, kx:kx + 16],
                                     start=(t == 0), stop=(t == 8))
                if xh_sb[p] is None:
                    xh_sb[p] = sbA.tile([128, 1024], F32R, tag="xh", name="xh")
                nc.vector.tensor_copy(xh_sb[p][:, src * 512:(src + 1) * 512], CP)

            def qk_units(p, b):
                def one(which):
                    def u():
                        src = QSRC[b] if which == 0 else KSRC[b]
                        rhs = xh_sb[p][:, src * 512:src * 512 + 512]
                        PB = pwp.tile([128, 512], F32, tag="pw", name="PB")
                        nc.tensor.matmul(
                            out=PB,
                            lhsT=wcols(OFF_QK + which * 512 + b * 128, 128),
                            rhs=rhs, start=True, stop=True)
                        t = sbB.tile([128, 512], F32R,
                                     tag=f"{'qk'[which]}{b}",
                                     name=f"{'qk'[which]}{b}")
                        if p == 1 and which == 0:
                            nc.scalar.activation(out=t, in_=PB, func=AF.Copy)
                        else:
                            nc.vector.tensor_copy(t, PB)
                        if which == 0:
                            q_sb[p][b] = t
                        else:
                            k_sb[p][b] = t
                    return u
                return [one(0), one(1)]

            def vt_units(p, src):
                """vt[d, (sc), (g), (v32|ones32)] for branches (src, src+2)."""
                def one(sc):
                    def u():
                        if sc == 0:
                            for j in range(2):
                                b = src + 2 * j
                                vt_sb[p][b] = sbB.tile(
                                    [128, 4, 4, 64], BF16,
                                    tag=f"vt{b}", name=f"vt{b}")
                                nc.gpsimd.memset(
                                    vt_sb[p][b][:, :, :, 32:64], 1.0)
                        s, c = divmod(sc, 2)
                        VT = pwp.tile([128, 256], F32, tag="pw", name="VT")
                        nc.tensor.matmul(
                            out=VT,
                            lhsT=xh_sb[p][:, src * 512 + s * 256 + c * 128:
                                          src * 512 + s * 256 + c * 128 + 128],
                            rhs=wcols(OFF_WV + src * 256, 256),
                            start=True, stop=True)
                        vv = VT.rearrange("p (j g d) -> p j g d", j=2, g=4)
                        for j in range(2):
                            b = src + 2 * j
                            if p == 0:
                                nc.vector.tensor_copy(
                                    vt_sb[p][b][:, sc, :, 0:32], vv[:, j, :, :])
                            else:
                                nc.scalar.activation(
                                    out=vt_sb[p][b][:, sc, :, 0:32],
                                    in_=vv[:, j, :, :], func=AF.Copy)
                    return u
                return [one(sc) for sc in range(4)]

            def emit_cprev(p):
                cprev_sb[p] = sbA.tile([128, 512], F32, tag="cprev", name="cprev")
                nc.sync.dma_start(
                    out=cprev_sb[p],
                    in_=cin[2 * p:2 * p + 2].rearrange("s c q -> c s q"))

            # ---------------- attention iteration pieces ----------------
            def emit_scores_exp(p, b, s, seps):
                """scores + exp; seps = 4 lists of full-row matmul thunks
                emitted at the row-group boundaries (after g0/g1/g2/g3)."""
                kv = k_sb[p][b].rearrange("p (s c d) -> p s c d", s=2, c=2)
                qv = q_sb[p][b].rearrange("p (s q) -> p s q", s=2)
                pT = ptp.tile([128, 2048], BF16, tag="pt", name="pT")
                for h in range(2):
                    ST = stp.tile([128, 1024], F32, tag="st", name="ST")
                    for gg in range(2):
                        g = 2 * h + gg
                        for c in range(2):
                            nc.tensor.matmul(
                                out=ST[:, gg * 512 + c * 256:gg * 512 + c * 256 + 256],
                                lhsT=kv[32 * g:32 * g + 32, s, c, :],
                                rhs=qv[32 * g:32 * g + 32, s, :],
                                start=True, stop=True, skip_group_check=True,
                                tile_position=(32 * g, 0))
                        for t in seps[g]:
                            t()
                    nc.scalar.activation(out=pT[:, h * 1024:(h + 1) * 1024],
                                         in_=ST, func=AF.Exp)
                return pT

            def make_pvz(p, b, s, pT, pool=None):
                """AZ [64, 1024] = per head g: [a_g; Z_g] in col block g.
                Returns 8 full-row matmul thunks (vt looked up at run time)."""
                pool = pool or azp
                AZ = pool.tile([64, 1024], F32,
                               tag="az" if pool is azp else "st", name="AZ")
                thunks = []
                for g in range(4):
                    for c in range(2):
                        def pvmm(g=g, c=c):
                            vt = vt_sb[p][b]
                            nc.tensor.matmul(
                                out=AZ[0:64, g * 256:(g + 1) * 256],
                                lhsT=vt[:, s * 2 + c, g, :],
                                rhs=pT[:, g * 512 + c * 256:g * 512 + c * 256 + 256],
                                start=(c == 0), stop=(c == 1),
                                skip_group_check=True)
                        thunks.append(pvmm)
                return AZ, thunks

            # mixed norm-group sizes: big groups early (fewer restack DMA
            # bursts), small groups at the end (short tail chains)
            GSTART = [0, 4, 8, 12, 14]
            GLEN = [4, 4, 4, 2, 2]
            IT2G = {}
            for gid, (st0, ln) in enumerate(zip(GSTART, GLEN)):
                for sl in range(ln):
                    IT2G[st0 + sl] = (gid, sl)

            def emit_azcopy(it, AZ, grp_tiles):
                """copy AZ psum into this group's a_flat slice [64, 1024],
                split across DVE and ACT to halve the copy latency."""
                gid, slot = IT2G[it]
                if grp_tiles[gid] is None:
                    grp_tiles[gid] = afp.tile([64, GLEN[gid] * 1024], F32,
                                              tag="aflat", name="a_flat")
                af = grp_tiles[gid]
                nc.vector.tensor_copy(
                    af[:, slot * 1024:slot * 1024 + 512], AZ[:, 0:512])
                nc.scalar.activation(
                    out=af[:, slot * 1024 + 512:slot * 1024 + 1024],
                    in_=AZ[:, 512:1024], func=AF.Copy)

            def emit_group_norm(gid, grp_tiles):
                """restack a group's a/Z via 8 DMAs, then one batched
                reciprocal (DVE) and one multiply."""
                st0, ln = GSTART[gid], GLEN[gid]
                p = st0 // 8
                af = grp_tiles[gid]
                afv = af.rearrange("p (it g q) -> p it g q", it=ln, g=4)
                a_tmp = sbN.tile([128, 4, 256], F32R, tag=f"atmp{gid % 2}",
                                 name="a_tmp")[:, 0:ln, :]
                last_atmp[0] = a_tmp
                zb = sbN.tile([128, 4, 256], F32, tag="zb", name="zb")[:, 0:ln, :]
                for g in range(4):
                    nc.sync.dma_start(
                        out=a_tmp[32 * g:32 * g + 32, :, :],
                        in_=afv[0:32, :, g, :].bitcast(F32R))
                    nc.sync.dma_start(
                        out=zb[32 * g:32 * g + 32, :, :],
                        in_=afv[32:64, :, g, :])
                rz = sbN.tile([128, 4, 256], F32, tag="rz", name="rz")[:, 0:ln, :]
                nc.vector.reciprocal_approx_fast(out=rz, in_=zb)
                if a_all[p] is None:
                    a_all[p] = sbA.tile([128, 2048], F32R, tag="aall",
                                        name="a_all")
                col = (st0 % 8) * 256
                eng = nc.vector if gid >= 2 else nc.gpsimd
                eng.tensor_mul(
                    a_all[p][:, col:col + ln * 256],
                    a_tmp.rearrange("p it q -> p (it q)").bitcast(F32),
                    rz.rearrange("p it q -> p (it q)"))

            # ---------------- gates / state / output ----------------
            def emit_gate(p, gi, pool=None):
                # a_all slot order is iteration order: (branch BORDER[j], s).
                # Skips first (ready immediately), then token matmuls in slot
                # order -- each becomes ready as its norm group completes, so
                # the list scheduler starts gates before all groups land.
                pool = pool or pwp
                G = pool.tile([128, 512], F32,
                              tag="pw" if pool is pwp else "st", name="G")
                av = a_all[p].rearrange("p (j s q) -> p j (s q)", j=4, s=2)
                for pos, b in enumerate(BORDER):
                    nc.tensor.matmul(out=G,
                                     lhsT=wcols(OFF_TOK + (gi * 4 + b) * 128, 128),
                                     rhs=av[:, pos, :],
                                     start=(pos == 0), stop=False)
                nc.tensor.matmul(out=G, lhsT=wcols(OFF_SKIP + (gi * 2) * 128, 128),
                                 rhs=xh_sb[p][:, 0:512], start=False, stop=False)
                nc.tensor.matmul(out=G, lhsT=wcols(OFF_SKIP + (gi * 2 + 1) * 128, 128),
                                 rhs=xh_sb[p][:, 512:1024], start=False, stop=True)
                gate_sb[p][gi] = sbA.tile([128, 512], F32, tag=f"gate{gi}",
                                          name=f"gate{gi}")
                scale = 1.0 if gi == 2 else 0.5
                nc.scalar.activation(out=gate_sb[p][gi], in_=G, func=AF.Tanh,
                                     bias=btok_ap[:, gi:gi + 1], scale=scale)

            def emit_update_out(p):
                # pass 0 math on the idle Pool engine, pass 1 on DVE (short tail)
                eng = nc.gpsimd if p == 0 else nc.vector
                ti, tf, gg_, to = gate_sb[p]
                def fixup(t):
                    # sigmoid(x) = 0.5*tanh(x/2) + 0.5; t = tanh(x/2 + b/2)
                    eng.tensor_scalar(out=t, in0=t, scalar1=0.5, scalar2=0.5,
                                      op0=ALU.mult, op1=ALU.add)
                    return t
                i_ = fixup(ti)
                f_ = fixup(tf)
                o_ = fixup(to)
                fc = sbA.tile([128, 512], F32, tag="fc", name="fc")
                eng.tensor_mul(fc, f_, cprev_sb[p])
                ig = sbA.tile([128, 512], F32, tag="ig", name="ig")
                eng.tensor_mul(ig, i_, gg_)
                eng.tensor_add(fc, fc, ig)            # c state
                nc.scalar.activation(out=ig, in_=fc, func=AF.Tanh)
                hs = sbA.tile([128, 512], F32R, tag="hs", name="hs")
                nc.vector.tensor_mul(hs, o_, ig)      # h_new
                OUT = pwp.tile([128, 512], F32, tag="pw", name="OUT")
                nc.tensor.matmul(out=OUT, lhsT=wcols(OFF_WOUT, 128),
                                 rhs=hs, start=True, stop=True)
                nc.vector.tensor_scalar_add(ig, OUT, bout_ap[:, 0:1])
                nc.sync.dma_start(
                    out=yout[2 * p:2 * p + 2].rearrange("s c q -> c s q"),
                    in_=ig.rearrange("p (s q) -> p s q", s=2))

            def dummy_sep():
                # tiny full-row (K=128) matmul used as a row-group separator
                d = pwp.tile([32, 32], F32, tag="pw", name="dsep")
                nc.tensor.matmul(out=d, lhsT=ones32, rhs=ones32,
                                 start=True, stop=True, skip_group_check=True)

            def emit_warm(n):
                # dependency-free matmuls that keep the PE p-state high while
                # the engine would otherwise idle waiting on a cross-engine
                # chain (the cost model halves PE clock after idle periods)
                for _ in range(n):
                    d = pwp.tile([32, 512], F32, tag="pw", name="warm")
                    nc.tensor.matmul(out=d, lhsT=ones32,
                                     rhs=wcols(OFF_CONVH, 512),
                                     start=True, stop=True,
                                     skip_group_check=True)

            # ---------------- emission schedule ----------------
            # prologue: all input DMAs + both passes' input stages upfront so
            # no input DMA ever queues behind a compute-dependent restack DMA
            # on the in-order SP queue.
            emit_input_dmas(0)
            nc.sync.dma_start(out=wb[:, WCH0:WCH1],
                              in_=wblob[:, WCH0:WCH1].bitcast(F32R))
            nc.sync.dma_start(out=wb[:, WCH1:WCH2],
                              in_=wblob[:, WCH1:WCH2].bitcast(F32R))
            emit_input_dmas(1)
            nc.sync.dma_start(out=wb[:, WCH2:WCH3],
                              in_=wblob[:, WCH2:WCH3].bitcast(F32R))
            emit_cprev(0)
            emit_cprev(1)
            emit_xt(0)
            emit_conv(0, 1)              # hc pass0
            for u in qk_units(0, 3):
                u()
            for u in vt_units(0, 1):     # vT for b1, b3 (hc source)
                u()

            # Remaining prep work goes into a FIFO of self-contained units
            # (matmul + its PSUM-draining copy). Units are drained through
            # the score-group separator slots so their copy latency hides
            # behind score/pvz matmuls instead of stalling the in-order PE.
            from collections import deque
            fq = deque()
            fq.append(lambda: emit_conv(0, 0))
            fq.extend(qk_units(0, 1))
            fq.extend(qk_units(0, 2))
            fq.extend(qk_units(0, 0))
            fq.extend(vt_units(0, 0))
            fq.append(lambda: emit_xt(1))
            fq.append(lambda: emit_conv(1, 1))
            fq.append(lambda: emit_conv(1, 0))
            fq.extend(qk_units(1, 3))
            fq.extend(vt_units(1, 1))
            fq.extend(qk_units(1, 1))
            fq.extend(qk_units(1, 2))
            fq.extend(qk_units(1, 0))
            fq.extend(vt_units(1, 0))

            # pvz of iteration i is interleaved into scores of iteration i+2,
            # giving its azcopy a full iteration window before pvz_{i+1}
            # reuses the single AZ PSUM buffer.
            iters = [(p, b, s) for p in (0, 1) for b in BORDER for s in (0, 1)]
            grp_tiles = [None] * 5
            GEND = {3: 0, 7: 1, 11: 2, 13: 3, 15: 4}
            last_atmp = [None]
            pend = []
            for i, (p, b, s) in enumerate(iters):
                while fq and (q_sb[p][b] is None or k_sb[p][b] is None
                              or (len(pend) == 2
                                  and vt_sb[pend[0][3]][pend[0][4]] is None)):
                    fq.popleft()()
                if len(pend) < 2:
                    seps = [[fq.popleft()] if fq else [dummy_sep]
                            for _ in range(4)]
                else:
                    th = pend[0][1]
                    seps = [th[0:2], th[2:4], th[4:6], th[6:8]]
                    for g in (1, 3):
                        if fq:
                            seps[g] = list(seps[g]) + [fq.popleft()]
                pT = emit_scores_exp(p, b, s, seps)
                if len(pend) == 2:
                    j, _, AZj, _, _ = pend.pop(0)
                    emit_azcopy(j, AZj, grp_tiles)
                    if j in GEND:
                        emit_group_norm(GEND[j], grp_tiles)
                # last two iterations borrow the (dead by then) ST buffers so
                # their PV matmuls don't serialize on the single AZ buffer
                AZ, thunks = make_pvz(p, b, s, pT,
                                      pool=stp if i >= 14 else azp)
                pend.append((i, thunks, AZ, p, b))
                while fq and i >= 13:
                    fq.popleft()()
            for j, th, AZj, _, _ in pend:
                for t in th:
                    t()
                emit_azcopy(j, AZj, grp_tiles)
                if j in GEND:
                    emit_group_norm(GEND[j], grp_tiles)
            for gi in range(4):
                emit_gate(0, gi)
            emit_gate(1, 0, stp)
            emit_gate(1, 1, stp)
            emit_gate(1, 2, pwp)
            emit_gate(1, 3, pwp)
            emit_update_out(0)
            emit_update_out(1)
            # a few warmers pinned (via their rhs, the last group's restacked
            # a_tmp) to the final norm window so the PE clock stays high right
            # before the pass-1 gate matmuls
            for _ in range(14):
                d = pwp.tile([32, 512], F32, tag="pw", name="warm")
                nc.tensor.matmul(out=d, lhsT=ones32,
                                 rhs=last_atmp[0].rearrange("p it q -> p (it q)"),
                                 start=True, stop=True, skip_group_check=True)

    nc.compile()
    return nc


def _prep_shared(inputs):
    f = np.float32
    c = np.ascontiguousarray
    W_cx, W_ch = np.asarray(inputs["W_cx"], f), np.asarray(inputs["W_ch"], f)
    W_q, W_k, W_v = (np.asarray(inputs[k], f) for k in ("W_q", "W_k", "W_v"))
    W_tok, W_skip = np.asarray(inputs["W_tok"], f), np.asarray(inputs["W_skip"], f)

    blob = np.zeros((128, NWCOL), f)
    blob[0:64, OFF_WIN:OFF_WIN + 128] = np.asarray(inputs["W_in"], f).T
    blob[:, OFF_BIN] = np.asarray(inputs["b_in"], f)
    # conv taps: [c, tap, o]
    blob[:, OFF_CONVH:OFF_CONVH + 1152] = \
        W_ch.transpose(1, 2, 3, 0).reshape(128, 1152)
    blob[:, OFF_CONVX:OFF_CONVX + 1152] = \
        W_cx.transpose(1, 2, 3, 0).reshape(128, 1152)
    # q then k: [c, branch*128]
    blob[:, OFF_QK:OFF_QK + 512] = W_q.transpose(2, 0, 1).reshape(128, 512)
    blob[:, OFF_QK + 512:OFF_QK + 1024] = W_k.transpose(2, 0, 1).reshape(128, 512)
    # wv: [c, src, branch-pair]: xc feeds branches (0,2), hc feeds (1,3)
    blob[:, OFF_WV:OFF_WV + 256] = np.concatenate([W_v[0].T, W_v[2].T], axis=1)
    blob[:, OFF_WV + 256:OFF_WV + 512] = np.concatenate([W_v[1].T, W_v[3].T], axis=1)
    blob[:, OFF_ONES:OFF_ONES + 32] = 1.0
    # tok: [a, gate*4 + branch]
    blob[:, OFF_TOK:OFF_TOK + 2048] = W_tok.transpose(3, 0, 1, 2).reshape(128, 2048)
    # skip: [c, gate*2 + src]
    blob[:, OFF_SKIP:OFF_SKIP + 1024] = \
        W_skip.transpose(3, 0, 1, 2).reshape(128, 1024)
    blob[:, OFF_WOUT:OFF_WOUT + 128] = np.asarray(inputs["W_out"], f).T
    btok = np.asarray(inputs["b_tok"], f).T              # [R, 4]
    sc = np.array([0.5, 0.5, 1.0, 0.5], f)[None, :]
    blob[:, OFF_BTOK:OFF_BTOK + 4] = btok * sc
    blob[:, OFF_BOUT] = np.asarray(inputs["b_out"], f)
    return {"wblob": c(blob)}


def kernel(**inputs):
    from concourse.bass_utils import run_bass_kernel_spmd
    if "nc" not in _CACHE:
        _CACHE["nc"] = _build_program()
    nc = _CACHE["nc"]
    f = np.float32
    x = np.asarray(inputs["x"], f).reshape(N, I, HW)
    hp = np.asarray(inputs["h_prev"], f).reshape(N, R, HW)
    cp = np.asarray(inputs["c_prev"], f).reshape(N, R, HW)
    shared = _prep_shared(inputs)
    in_maps = []
    for ci in range(NCORES):
        sl = slice(S * ci, S * ci + S)
        m = dict(shared)
        m["xin"] = np.ascontiguousarray(x[sl])
        m["hin"] = np.ascontiguousarray(hp[sl])
        m["cin"] = np.ascontiguousarray(cp[sl])
        in_maps.append(m)
    res = run_bass_kernel_spmd(nc, in_maps, core_ids=list(range(NCORES)))
    y = np.concatenate([r["yout"].reshape(S, R, H, W) for r in res.results],
                       axis=0)
    return y.astype(np.float32)


# revision 82
# speedup vs baseline: 1.0426x; 1.0426x over previous
# Trainium2 Bass kernel for nn_CvtLstm: ConvLSTM cell with 4-branch,
# 4-head spatial attention. Data-parallel over batch N=32 across 8
# NeuronCores (4 samples per core); weights replicated to every core.
#
# Per-core layout: channels on partitions, flattened 16x16 spatial (256)
# on the free dim. conv3x3 = 9 shifted matmuls reading a zero-padded
# [128, 2, 18, 18] tile. Attention scores are computed directly in the
# transposed [d, q] layout (lhsT = per-head k rows, K=32 row-partial
# matmuls); exp on the ACT engine with no max subtraction (scores lie in
# [-9, 8]), written as bf16; the PV product and the softmax denominator Z
# come from one M=64 matmul per (head, d-chunk) whose bf16 weight columns
# are [vT_g | ones]. Matmul outputs must start at partition 0, so heads
# are separated via a DMA restack batched over norm groups (4 iterations
# early, 2 at the end: 8 DMAs + one batched reciprocal + one multiply per
# group). Each AZ PSUM->SBUF drain is split across DVE and ACT.
#
# The Tile framework list-schedules instructions by readiness with
# emission order as priority, so the emission schedule is shaped to give
# it slack: pvz matmuls for iteration i are interleaved into the score
# row-group boundaries of iteration i+2 (full-row separators for the
# row-partial score matmuls, and a full iteration window for the AZ
# drain); prep work (convs / qk / vt projections) is queued as
# self-contained matmul+copy units drained through those boundaries; the
# last two iterations' PV outputs borrow the then-dead ST buffers; gate
# matmul operand order makes each gate accumulate as its norm group
# lands, with four concurrent PSUM accumulators for the final gates; a
# few dependency-pinned warmup matmuls hold the PE p-state through the
# final norm chain.
#
# Gate sigmoids are computed as sigmoid(x) = 0.5*(1 + tanh(x/2)) so every
# activation (tanh/exp) lives in one ACT function table -- no table swaps.
# All weights arrive in one packed DRAM blob via 4 chunked DMAs (a tiny
# first chunk unblocks the input projection immediately) and every input
# DMA is issued before any compute-dependent DMA so the in-order SP queue
# never head-of-line blocks. Zero padding and the vt ones-columns are
# written by Pool-engine memsets. Elementwise work is spread over
# Pool/DVE/ACT to keep each engine under the PE roofline.
# TimelineSim: 103.7 us/core (baseline of this kernel's ancestor: 141.9).

import numpy as np

N, I, H, W = 32, 64, 16, 16
R, CM, A, HEADS, HC = 128, 128, 128, 4, 32
HW = H * W           # 256
S = 4                # samples per core
NCORES = 8

# weight blob column offsets
OFF_WIN = 0          # [64p, 128] (rows 64:128 zero)
OFF_BIN = 128        # [128, 1]
OFF_CONVH = 129      # [128, 9*128] W_ch taps
OFF_CONVX = OFF_CONVH + 1152   # [128, 9*128] W_cx taps
OFF_QK = OFF_CONVX + 1152      # [128, 2*4*128] (q|k, branch)
OFF_WV = OFF_QK + 1024         # [128, 2*256] (src, branch-pair)
OFF_ONES = OFF_WV + 512        # [128, 32]
OFF_TOK = OFF_ONES + 32        # [128, 4*4*128] (gate, branch)
OFF_SKIP = OFF_TOK + 2048      # [128, 4*2*128] (gate, src)
OFF_WOUT = OFF_SKIP + 1024     # [128, 128]
OFF_BTOK = OFF_WOUT + 128      # [128, 4] (pre-scaled 0.5 except gate 2)
OFF_BOUT = OFF_BTOK + 4        # [128, 1]
NWCOL = OFF_BOUT + 1

# chunk boundaries for the 4 weight DMAs (ordered by first use)
WCH0 = OFF_CONVH            # winT + b_in (tiny -- unblocks XT fast)
WCH1 = OFF_CONVX            # conv_h
WCH2 = OFF_TOK              # conv_x + qk + wv + ones
WCH3 = NWCOL                # tok + skip + wout + btok + bout

_CACHE = {}


def _build_program():
    import contextlib
    import concourse.bacc as bacc
    import concourse.mybir as mybir
    import concourse.tile as tile
    import concourse.bass as bass

    F32 = mybir.dt.float32
    F32R = mybir.dt.float32r
    BF16 = mybir.dt.bfloat16
    AF = mybir.ActivationFunctionType
    ALU = mybir.AluOpType

    nc = bacc.Bacc("TRN2", target_bir_lowering=False, debug=False)

    def dram(name, shape, kind="ExternalInput"):
        return nc.dram_tensor(name, list(shape), F32, kind=kind).ap()

    xin = dram("xin", [S, I, HW])
    hin = dram("hin", [S, R, HW])
    cin = dram("cin", [S, R, HW])
    wblob = dram("wblob", [128, NWCOL])
    yout = dram("yout", [S, R, HW], kind="ExternalOutput")

    QSRC = [0, 0, 1, 1]   # q source per branch: 0=xc, 1=hc
    KSRC = [0, 1, 0, 1]   # k/v source per branch
    BORDER = [3, 1, 2, 0]  # per-pass branch order (b3 = pure hc, earliest)

    with tile.TileContext(nc) as tc:
        with contextlib.ExitStack() as ctx:
            wpool = ctx.enter_context(tc.tile_pool(name="wts", bufs=1))
            sbA = ctx.enter_context(tc.tile_pool(name="sbA", bufs=2))
            sbB = ctx.enter_context(tc.tile_pool(name="sbB", bufs=2))
            sbN = ctx.enter_context(tc.tile_pool(name="sbN", bufs=1))
            ptp = ctx.enter_context(tc.tile_pool(name="ptp", bufs=3))
            afp = ctx.enter_context(tc.tile_pool(name="afp", bufs=2))
            stp = ctx.enter_context(tc.tile_pool(name="st", bufs=2, space="PSUM"))
            azp = ctx.enter_context(tc.tile_pool(name="az", bufs=1, space="PSUM"))
            pwp = ctx.enter_context(tc.tile_pool(name="pw", bufs=2, space="PSUM"))

            # ---------------- weights to SBUF (one tile, 4 DMAs) ----------
            # tiny chunk 0 (win+b_in) first so XT can start immediately;
            # remaining chunks interleaved with the input DMAs below.
            wb = wpool.tile([128, NWCOL], F32R, tag="wb", name="wb")
            nc.sync.dma_start(out=wb[:, 0:WCH0],
                              in_=wblob[:, 0:WCH0].bitcast(F32R))

            def wcols(off, n):
                return wb[:, off:off + n]

            ones32 = wcols(OFF_ONES, 32)
            b_in_ap = wcols(OFF_BIN, 1).bitcast(F32)
            btok_ap = wcols(OFF_BTOK, 4).bitcast(F32)
            bout_ap = wcols(OFF_BOUT, 1).bitcast(F32)

            # ---------------- per-pass state ----------------
            pad_sb = [None, None]      # [128, 2, 648]: 0=xt pad, 1=h pad
            xh_sb = [None, None]       # [128, 1024]: xc | hc
            q_sb = [[None] * 4, [None] * 4]
            k_sb = [[None] * 4, [None] * 4]
            vt_sb = [[None] * 4, [None] * 4]   # per branch: [128, 4, 4, 64]
            a_all = [None, None]
            cprev_sb = [None, None]
            gate_sb = [[None] * 4, [None] * 4]

            x2_sb = [None, None]

            def emit_input_dmas(p):
                """zero-padded xt/h pads (Pool memset) + h/x input DMAs."""
                pad = sbA.tile([128, 2, 648], F32R, tag="padc", name="pad")
                pad_sb[p] = pad
                nc.gpsimd.memset(pad.bitcast(F32), 0.0)
                x2_sb[p] = sbA.tile([64, 2, 256], F32R, tag="x2", name="x2")
                hv = pad[:, 1, :].rearrange("p (s y x) -> p s y x", s=2, y=18, x=18)
                for s in range(2):
                    nc.sync.dma_start(
                        out=hv[:, s, 1:17, 1:17],
                        in_=hin[2 * p + s].rearrange(
                            "c (h w) -> c h w", h=16).bitcast(F32R))
                nc.sync.dma_start(
                    out=x2_sb[p],
                    in_=xin[2 * p:2 * p + 2].rearrange("s c q -> c s q").bitcast(F32R))

            def emit_xt(p):
                """XT matmul + tanh into the xt pad."""
                pad = pad_sb[p]
                XT = pwp.tile([128, 512], F32, tag="pw", name="XT")
                nc.tensor.matmul(out=XT, lhsT=wb[0:64, OFF_WIN:OFF_WIN + 128],
                                 rhs=x2_sb[p].rearrange("p s q -> p (s q)"),
                                 start=True, stop=True)
                xv = pad[:, 0, :].rearrange("p (s y x) -> p s y x", s=2, y=18, x=18)
                nc.scalar.activation(
                    out=xv[:, :, 1:17, 1:17],
                    in_=XT.rearrange("p (s h w) -> p s h w", s=2, h=16, w=16),
                    func=AF.Tanh, bias=b_in_ap)

            def emit_conv(p, src):
                """3x3 SAME conv via 9 shifted matmuls; src 0=xc, 1=hc."""
                CP = pwp.tile([128, 512], F32, tag="pw", name="CP")
                pv = pad_sb[p][:, src, :].rearrange(
                    "p (s y x) -> p s y x", s=2, y=18, x=18)
                woff = OFF_CONVX if src == 0 else OFF_CONVH
                for t in range(9):
                    ky, kx = divmod(t, 3)
                    nc.tensor.matmul(out=CP, lhsT=wcols(woff + t * 128, 128),
                                     rhs=pv[:, :, ky:ky + 16, kx:kx + 16],
                                     start=(t == 0), stop=(t == 8))
                if xh_sb[p] is None:
                    xh_sb[p] = sbA.tile([128, 1024], F32R, tag="xh", name="xh")
                nc.vector.tensor_copy(xh_sb[p][:, src * 512:(src + 1) * 512], CP)

            def qk_units(p, b):
                def one(which):
                    def u():
                        src = QSRC[b] if which == 0 else KSRC[b]
                        rhs = xh_sb[p][:, src * 512:src * 512 + 512]
                        PB = pwp.tile([128, 512], F32, tag="pw", name="PB")
                        nc.tensor.matmul(
                            out=PB,
                            lhsT=wcols(OFF_QK + which * 512 + b * 128, 128),
                            rhs=rhs, start=True, stop=True)
                        t = sbB.tile([128, 512], F32R,
                                     tag=f"{'qk'[which]}{b}",
                                     name=f"{'qk'[which]}{b}")
                        if p == 1 and which == 0:
                            nc.scalar.activation(out=t, in_=PB, func=AF.Copy)
                        else:
                            nc.vector.tensor_copy(t, PB)
                        if which == 0:
                            q_sb[p][b] = t
                        else:
                            k_sb[p][b] = t
                    return u
                return [one(0), one(1)]

            def vt_units(p, src):
                """vt[d, (sc), (g), (v32|ones32)] for branches (src, src+2)."""
                def one(sc):
                    def u():
                        if sc == 0:
                            for j in range(2):
                                b = src + 2 * j
                                vt_sb[p][b] = sbB.tile(
                                    [128, 4, 4, 64], BF16,
                                    tag=f"vt{b}", name=f"vt{b}")
                                nc.gpsimd.memset(
                                    vt_sb[p][b][:, :, :, 32:64], 1.0)
                        s, c = divmod(sc, 2)
                        VT = pwp.tile([128, 256], F32, tag="pw", name="VT")
                        nc.tensor.matmul(
                            out=VT,
                            lhsT=xh_sb[p][:, src * 512 + s * 256 + c * 128:
                                          src * 512 + s * 256 + c * 128 + 128],
                            rhs=wcols(OFF_WV + src * 256, 256),
                            start=True, stop=True)
                        vv = VT.rearrange("p (j g d) -> p j g d", j=2, g=4)
                        for j in range(2):
                            b = src + 2 * j
                            if p == 0:
                                nc.vector.tensor_copy(
                                    vt_sb[p][b][:, sc, :, 0:32], vv[:, j, :, :])
                            else:
                                nc.scalar.activation(
                                    out=vt_sb[p][b][:, sc, :, 0:32],
                                    in_=vv[:, j, :, :], func=AF.Copy)
                    return u
                return [one(sc) for sc in range(4)]

            def emit_cprev(p):
                cprev_sb[p] = sbA.tile([128, 512], F32, tag="cprev", name="cprev")
                nc.sync.dma_start(
                    out=cprev_sb[p],
                    in_=cin[2 * p:2 * p + 2].rearrange("s c q -> c s q"))

            # ---------------- attention iteration pieces ----------------
            def emit_scores_exp(p, b, s, seps):
                """scores + exp; seps = 4 lists of full-row matmul thunks
                emitted at the row-group boundaries (after g0/g1/g2/g3)."""
                kv = k_sb[p][b].rearrange("p (s c d) -> p s c d", s=2, c=2)
                qv = q_sb[p][b].rearrange("p (s q) -> p s q", s=2)
                pT = ptp.tile([128, 2048], BF16, tag="pt", name="pT")
                for h in range(2):
                    ST = stp.tile([128, 1024], F32, tag="st", name="ST")
                    for gg in range(2):
                        g = 2 * h + gg
                        for c in range(2):
                            nc.tensor.matmul(
                                out=ST[:, gg * 512 + c * 256:gg * 512 + c * 256 + 256],
                                lhsT=kv[32 * g:32 * g + 32, s, c, :],
                                rhs=qv[32 * g:32 * g + 32, s, :],
                                start=True, stop=True, skip_group_check=True,
                                tile_position=(32 * g, 0))
                        for t in seps[g]:
                            t()
                    nc.scalar.activation(out=pT[:, h * 1024:(h + 1) * 1024],
                                         in_=ST, func=AF.Exp)
                return pT

            def make_pvz(p, b, s, pT, pool=None):
                """AZ [64, 1024] = per head g: [a_g; Z_g] in col block g.
                Returns 8 full-row matmul thunks (vt looked up at run time)."""
                pool = pool or azp
                AZ = pool.tile([64, 1024], F32,
                               tag="az" if pool is azp else "st", name="AZ")
                thunks = []
                for g in range(4):
                    for c in range(2):
                        def pvmm(g=g, c=c):
                            vt = vt_sb[p][b]
                            nc.tensor.matmul(
                                out=AZ[0:64, g * 256:(g + 1) * 256],
                                lhsT=vt[:, s * 2 + c, g, :],
                                rhs=pT[:, g * 512 + c * 256:g * 512 + c * 256 + 256],
                                start=(c == 0), stop=(c == 1),
                                skip_group_check=True)
                        thunks.append(pvmm)
                return AZ, thunks

            # mixed norm-group sizes: big groups early (fewer restack DMA
            # bursts), small groups at the end (short tail chains)
            GSTART = [0, 4, 8, 10, 12, 14]
            GLEN = [4, 4, 2, 2, 2, 2]
            IT2G = {}
            for gid, (st0, ln) in enumerate(zip(GSTART, GLEN)):
                for sl in range(ln):
                    IT2G[st0 + sl] = (gid, sl)

            def emit_azcopy(it, AZ, grp_tiles):
                """copy AZ psum into this group's a_flat slice [64, 1024],
                split across DVE and ACT to halve the copy latency."""
                gid, slot = IT2G[it]
                if grp_tiles[gid] is None:
                    grp_tiles[gid] = afp.tile([64, GLEN[gid] * 1024], F32,
                                              tag="aflat", name="a_flat")
                af = grp_tiles[gid]
                nc.vector.tensor_copy(
                    af[:, slot * 1024:slot * 1024 + 512], AZ[:, 0:512])
                nc.scalar.activation(
                    out=af[:, slot * 1024 + 512:slot * 1024 + 1024],
                    in_=AZ[:, 512:1024], func=AF.Copy)

            def emit_group_norm(gid, grp_tiles):
                """restack a group's a/Z via 8 DMAs, then one batched
                reciprocal (DVE) and one multiply."""
                st0, ln = GSTART[gid], GLEN[gid]
                p = st0 // 8
                af = grp_tiles[gid]
                afv = af.rearrange("p (it g q) -> p it g q", it=ln, g=4)
                a_tmp = sbN.tile([128, 4, 256], F32R, tag=f"atmp{gid % 2}",
                                 name="a_tmp")[:, 0:ln, :]
                last_atmp[0] = a_tmp
                zb = sbN.tile([128, 4, 256], F32, tag="zb", name="zb")[:, 0:ln, :]
                for g in range(4):
                    nc.sync.dma_start(
                        out=a_tmp[32 * g:32 * g + 32, :, :],
                        in_=afv[0:32, :, g, :].bitcast(F32R))
                    nc.sync.dma_start(
                        out=zb[32 * g:32 * g + 32, :, :],
                        in_=afv[32:64, :, g, :])
                rz = sbN.tile([128, 4, 256], F32, tag="rz", name="rz")[:, 0:ln, :]
                nc.vector.reciprocal_approx_fast(out=rz, in_=zb)
                if a_all[p] is None:
                    a_all[p] = sbA.tile([128, 2048], F32R, tag="aall",
                                        name="a_all")
                col = (st0 % 8) * 256
                eng = nc.vector if gid >= 3 else nc.gpsimd
                eng.tensor_mul(
                    a_all[p][:, col:col + ln * 256],
                    a_tmp.rearrange("p it q -> p (it q)").bitcast(F32),
                    rz.rearrange("p it q -> p (it q)"))

            # ---------------- gates / state / output ----------------
            def emit_gate(p, gi, pool=None):
                # a_all slot order is iteration order: (branch BORDER[j], s).
                # Skips first (ready immediately), then token matmuls in slot
                # order -- each becomes ready as its norm group completes, so
                # the list scheduler starts gates before all groups land.
                pool = pool or pwp
                G = pool.tile([128, 512], F32,
                              tag="pw" if pool is pwp else "st", name="G")
                av = a_all[p].rearrange("p (j s q) -> p j (s q)", j=4, s=2)
                for pos, b in enumerate(BORDER):
                    nc.tensor.matmul(out=G,
                                     lhsT=wcols(OFF_TOK + (gi * 4 + b) * 128, 128),
                                     rhs=av[:, pos, :],
                                     start=(pos == 0), stop=False)
                nc.tensor.matmul(out=G, lhsT=wcols(OFF_SKIP + (gi * 2) * 128, 128),
                                 rhs=xh_sb[p][:, 0:512], start=False, stop=False)
                nc.tensor.matmul(out=G, lhsT=wcols(OFF_SKIP + (gi * 2 + 1) * 128, 128),
                                 rhs=xh_sb[p][:, 512:1024], start=False, stop=True)
                gate_sb[p][gi] = sbA.tile([128, 512], F32, tag=f"gate{gi}",
                                          name=f"gate{gi}")
                scale = 1.0 if gi == 2 else 0.5
                nc.scalar.activation(out=gate_sb[p][gi], in_=G, func=AF.Tanh,
                                     bias=btok_ap[:, gi:gi + 1], scale=scale)

            def emit_update_out(p):
                # pass 0 math on the idle Pool engine, pass 1 on DVE (short tail)
                eng = nc.gpsimd if p == 0 else nc.vector
                ti, tf, gg_, to = gate_sb[p]
                def fixup(t):
                    # sigmoid(x) = 0.5*tanh(x/2) + 0.5; t = tanh(x/2 + b/2)
                    eng.tensor_scalar(out=t, in0=t, scalar1=0.5, scalar2=0.5,
                                      op0=ALU.mult, op1=ALU.add)
                    return t
                i_ = fixup(ti)
                f_ = fixup(tf)
                if p == 1:
                    # run the o fixup on the idle Pool, in parallel with the
                    # DVE state chain it would otherwise serialize behind
                    nc.gpsimd.tensor_scalar(out=to, in0=to, scalar1=0.5,
                                            scalar2=0.5, op0=ALU.mult,
                                            op1=ALU.add)
                    o_ = to
                else:
                    o_ = fixup(to)
                fc = sbA.tile([128, 512], F32, tag="fc", name="fc")
                eng.tensor_mul(fc, f_, cprev_sb[p])
                ig = sbA.tile([128, 512], F32, tag="ig", name="ig")
                eng.tensor_mul(ig, i_, gg_)
                eng.tensor_add(fc, fc, ig)            # c state
                nc.scalar.activation(out=ig, in_=fc, func=AF.Tanh)
                hs = sbA.tile([128, 512], F32R, tag="hs", name="hs")
                nc.vector.tensor_mul(hs, o_, ig)      # h_new
                OUT = pwp.tile([128, 512], F32, tag="pw", name="OUT")
                nc.tensor.matmul(out=OUT, lhsT=wcols(OFF_WOUT, 128),
                                 rhs=hs, start=True, stop=True)
                if p == 0:
                    # keep pass-0's bias add off the DVE queue that pass-1's
                    # state chain runs on
                    nc.scalar.activation(out=ig, in_=OUT, func=AF.Copy,
                                         bias=0.0)
                    nc.gpsimd.tensor_scalar_add(ig, ig, bout_ap[:, 0:1])
                else:
                    nc.vector.tensor_scalar_add(ig, OUT, bout_ap[:, 0:1])
                nc.sync.dma_start(
                    out=yout[2 * p:2 * p + 2].rearrange("s c q -> c s q"),
                    in_=ig.rearrange("p (s q) -> p s q", s=2))

            def dummy_sep():
                # tiny full-row (K=128) matmul used as a row-group separator
                d = pwp.tile([32, 32], F32, tag="pw", name="dsep")
                nc.tensor.matmul(out=d, lhsT=ones32, rhs=ones32,
                                 start=True, stop=True, skip_group_check=True)

            def emit_warm(n):
                # dependency-free matmuls that keep the PE p-state high while
                # the engine would otherwise idle waiting on a cross-engine
                # chain (the cost model halves PE clock after idle periods)
                for _ in range(n):
                    d = pwp.tile([32, 512], F32, tag="pw", name="warm")
                    nc.tensor.matmul(out=d, lhsT=ones32,
                                     rhs=wcols(OFF_CONVH, 512),
                                     start=True, stop=True,
                                     skip_group_check=True)

            # ---------------- emission schedule ----------------
            # prologue: all input DMAs + both passes' input stages upfront so
            # no input DMA ever queues behind a compute-dependent restack DMA
            # on the in-order SP queue.
            emit_input_dmas(0)
            nc.sync.dma_start(out=wb[:, WCH0:WCH1],
                              in_=wblob[:, WCH0:WCH1].bitcast(F32R))
            nc.sync.dma_start(out=wb[:, WCH1:WCH2],
                              in_=wblob[:, WCH1:WCH2].bitcast(F32R))
            emit_input_dmas(1)
            nc.sync.dma_start(out=wb[:, WCH2:WCH3],
                              in_=wblob[:, WCH2:WCH3].bitcast(F32R))
            emit_cprev(0)
            emit_cprev(1)
            emit_xt(0)
            emit_conv(0, 1)              # hc pass0
            for u in qk_units(0, 3):
                u()
            for u in vt_units(0, 1):     # vT for b1, b3 (hc source)
                u()

            # Remaining prep work goes into a FIFO of self-contained units
            # (matmul + its PSUM-draining copy). Units are drained through
            # the score-group separator slots so their copy latency hides
            # behind score/pvz matmuls instead of stalling the in-order PE.
            from collections import deque
            fq = deque()
            fq.append(lambda: emit_conv(0, 0))
            fq.extend(qk_units(0, 1))
            fq.extend(qk_units(0, 2))
            fq.extend(qk_units(0, 0))
            fq.extend(vt_units(0, 0))
            fq.append(lambda: emit_xt(1))
            fq.append(lambda: emit_conv(1, 1))
            fq.append(lambda: emit_conv(1, 0))
            fq.extend(qk_units(1, 3))
            fq.extend(vt_units(1, 1))
            fq.extend(qk_units(1, 1))
            fq.extend(qk_units(1, 2))
            fq.extend(qk_units(1, 0))
            fq.extend(vt_units(1, 0))

            # pvz of iteration i is interleaved into scores of iteration i+2,
            # giving its azcopy a full iteration window before pvz_{i+1}
            # reuses the single AZ PSUM buffer.
            iters = [(p, b, s) for p in (0, 1) for b in BORDER for s in (0, 1)]
            grp_tiles = [None] * 6
            GEND = {3: 0, 7: 1, 9: 2, 11: 3, 13: 4, 15: 5}
            last_atmp = [None]
            pend = []
            for i, (p, b, s) in enumerate(iters):
                while fq and (q_sb[p][b] is None or k_sb[p][b] is None
                              or (len(pend) == 2
                                  and vt_sb[pend[0][3]][pend[0][4]] is None)):
                    fq.popleft()()
                if len(pend) < 2:
                    seps = [[fq.popleft()] if fq else [dummy_sep]
                            for _ in range(4)]
                else:
                    th = pend[0][1]
                    seps = [th[0:2], th[2:4], th[4:6], th[6:8]]
                    for g in (1, 3):
                        if fq:
                            seps[g] = list(seps[g]) + [fq.popleft()]
                pT = emit_scores_exp(p, b, s, seps)
                if len(pend) == 2:
                    j, _, AZj, _, _ = pend.pop(0)
                    emit_azcopy(j, AZj, grp_tiles)
                    if j in GEND:
                        emit_group_norm(GEND[j], grp_tiles)
                # last two iterations borrow the (dead by then) ST buffers so
                # their PV matmuls don't serialize on the single AZ buffer
                AZ, thunks = make_pvz(p, b, s, pT,
                                      pool=stp if i >= 14 else azp)
                pend.append((i, thunks, AZ, p, b))
                while fq and i >= 13:
                    fq.popleft()()
            for j, th, AZj, _, _ in pend:
                for t in th:
                    t()
                emit_azcopy(j, AZj, grp_tiles)
                if j in GEND:
                    emit_group_norm(GEND[j], grp_tiles)
            for gi in range(4):
                emit_gate(0, gi)
            emit_gate(1, 0, stp)
            emit_gate(1, 1, stp)
            emit_gate(1, 2, pwp)
            emit_gate(1, 3, pwp)
            emit_update_out(0)
            emit_update_out(1)
            # a few warmers pinned (via their rhs, the last group's restacked
            # a_tmp) to the final norm window so the PE clock stays high right
            # before the pass-1 gate matmuls
            for _ in range(14):
                d = pwp.tile([32, 512], F32, tag="pw", name="warm")
                nc.tensor.matmul(out=d, lhsT=ones32,
                                 rhs=last_atmp[0].rearrange("p it q -> p (it q)"),
                                 start=True, stop=True, skip_group_check=True)

    nc.compile()
    return nc


def _prep_shared(inputs):
    f = np.float32
    c = np.ascontiguousarray
    W_cx, W_ch = np.asarray(inputs["W_cx"], f), np.asarray(inputs["W_ch"], f)
    W_q, W_k, W_v = (np.asarray(inputs[k], f) for k in ("W_q", "W_k", "W_v"))
    W_tok, W_skip = np.asarray(inputs["W_tok"], f), np.asarray(inputs["W_skip"], f)

    blob = np.zeros((128, NWCOL), f)
    blob[0:64, OFF_WIN:OFF_WIN + 128] = np.asarray(inputs["W_in"], f).T
    blob[:, OFF_BIN] = np.asarray(inputs["b_in"], f)
    # conv taps: [c, tap, o]
    blob[:, OFF_CONVH:OFF_CONVH + 1152] = \
        W_ch.transpose(1, 2, 3, 0).reshape(128, 1152)
    blob[:, OFF_CONVX:OFF_CONVX + 1152] = \
        W_cx.transpose(1, 2, 3, 0).reshape(128, 1152)
    # q then k: [c, branch*128]
    blob[:, OFF_QK:OFF_QK + 512] = W_q.transpose(2, 0, 1).reshape(128, 512)
    blob[:, OFF_QK + 512:OFF_QK + 1024] = W_k.transpose(2, 0, 1).reshape(128, 512)
    # wv: [c, src, branch-pair]: xc feeds branches (0,2), hc feeds (1,3)
    blob[:, OFF_WV:OFF_WV + 256] = np.concatenate([W_v[0].T, W_v[2].T], axis=1)
    blob[:, OFF_WV + 256:OFF_WV + 512] = np.concatenate([W_v[1].T, W_v[3].T], axis=1)
    blob[:, OFF_ONES:OFF_ONES + 32] = 1.0
    # tok: [a, gate*4 + branch]
    blob[:, OFF_TOK:OFF_TOK + 2048] = W_tok.transpose(3, 0, 1, 2).reshape(128, 2048)
    # skip: [c, gate*2 + src]
    blob[:, OFF_SKIP:OFF_SKIP + 1024] = \
        W_skip.transpose(3, 0, 1, 2).reshape(128, 1024)
    blob[:, OFF_WOUT:OFF_WOUT + 128] = np.asarray(inputs["W_out"], f).T
    btok = np.asarray(inputs["b_tok"], f).T              # [R, 4]
    sc = np.array([0.5, 0.5, 1.0, 0.5], f)[None, :]
    blob[:, OFF_BTOK:OFF_BTOK + 4] = btok * sc
    blob[:, OFF_BOUT] = np.asarray(inputs["b_out"], f)
    return {"wblob": c(blob)}


def kernel(**inputs):
    from concourse.bass_utils import run_bass_kernel_spmd
    if "nc" not in _CACHE:
        _CACHE["nc"] = _build_program()
    nc = _CACHE["nc"]
    f = np.float32
    x = np.asarray(inputs["x"], f).reshape(N, I, HW)
    hp = np.asarray(inputs["h_prev"], f).reshape(N, R, HW)
    cp = np.asarray(inputs["c_prev"], f).reshape(N, R, HW)
    shared = _prep_shared(inputs)
    in_maps = []
    for ci in range(NCORES):
        sl = slice(S * ci, S * ci + S)
        m = dict(shared)
        m["xin"] = np.ascontiguousarray(x[sl])
        m["hin"] = np.ascontiguousarray(hp[sl])
        m["cin"] = np.ascontiguousarray(cp[sl])
        in_maps.append(m)
    res = run_bass_kernel_spmd(nc, in_maps, core_ids=list(range(NCORES)))
    y = np.concatenate([r["yout"].reshape(S, R, H, W) for r in res.results],
                       axis=0)
    return y.astype(np.float32)
